# revision 41
# baseline (speedup 1.0000x reference)
"""GATv2 (2-layer, PyG-style self-loops) on 8 Trainium2 NeuronCores — bf16.

No dma_gather: the host stages per-edge source features x[src] in edge
order (pure layout), and the device projects them per-edge (lhsT=xeT
tile, rhs=Wl) straight into the score PSUM. This removes the SWDGE Q7
descriptor-generation serial bottleneck (~1ms/layer) and the table-build
prologue of the gather-based design.

Sharding: dst nodes split across 8 cores (12500 each); edges routed to the
core owning dst. Nodes packed into SLOT-GROUPS of <=32 slots and <=512
edges; each group's edges fill 4 tiles of 128 rows ("phases" p=row//128).
All feature columns are stored c-major (c,h) so the score reduce over c
runs as contiguous column halvings on the DVE.

Per phase-tile (bf16, PSUM fp32):
  psz  - 8 proj matmuls (lhsT=xeT 128-col tile, rhs=Wl) write xl per edge
         into PSUM (one start=True per tile; start clears has_written
         bank-wide), a scalar-engine Copy extracts xl to SBUF (for the
         message; H=1 writes straight into the wp slots), then 8 band
         matmuls accumulate ee+xr+biases: lhsT=[eaT(16);Mt(32);evalid]
         band, rhs=[We;br;bl;xr_g] (weconst + per-group xr matmuls).
  z    - LeakyReLU on the scalar engine.
  p    - z*att (DVE), halving-tree reduce, Exp on scalar engine.
  out  - lhsT=[xl*p | p] (H=1: [xl | 1] with p folded into the scatter
         rhs M*p) contracted with the one-hot M into a 512-slot PSUM
         window, 4 phases per slice consecutively.
Finalize (emitted one window late so its cross-engine chain never blocks
queue heads): reciprocal_approx_fast of the denominators (+eps for pad
slots), matmul-expanded to [HC,512], normalize, bias (+bl since
sum-alpha=1) and ELU via scalar-engine Relu/Exp with bias APs; output
stays [HC, S] (host transposes).
"""

import numpy as np
import ml_dtypes

BF16 = ml_dtypes.bfloat16

N_NODES = 100000
D_EDGE = 16
H1, C1 = 8, 8
D_NODE = 128
D_EMB = 64
NEG_SLOPE = 0.2
N_CORES = 8
NPC = N_NODES // N_CORES          # 12500 dst nodes per core
SLOTS = 32                        # slots per group
EPT = 128                         # edge rows per phase-tile
NPH = 4                           # tiles (phases) per group
GEDGE = NPH * EPT                 # 512 edge rows per group
GPW = 16                          # groups per psum window (512 slots)


def _preprocess(edge_index, edge_attr):
    src = np.asarray(edge_index[0], dtype=np.int64)
    dst = np.asarray(edge_index[1], dtype=np.int64)
    ea = np.asarray(edge_attr, dtype=np.float32)

    deg = np.bincount(dst, minlength=N_NODES).astype(np.float32)
    order0 = np.argsort(dst, kind="stable")
    ds = dst[order0]
    bnd0 = np.flatnonzero(np.diff(ds)) + 1
    starts0 = np.concatenate([[0], bnd0])
    ea_sum = np.zeros((N_NODES, D_EDGE), np.float32)
    ea_sum[ds[starts0]] = np.add.reduceat(ea[order0], starts0, axis=0)
    ea_mean = ea_sum / np.maximum(deg, 1.0)[:, None]

    loop = np.arange(N_NODES, dtype=np.int64)
    src2 = np.concatenate([src, loop])
    dst2 = np.concatenate([dst, loop])
    ea2 = np.concatenate([ea, ea_mean], axis=0)

    cores = []
    for c in range(N_CORES):
        lo = c * NPC
        m = (dst2 >= lo) & (dst2 < lo + NPC)
        cores.append((src2[m], dst2[m] - lo, ea2[m]))

    # --- per-core grouping: <=32 slots/group, <=512 edges/group (FFD) ---
    packed = []
    for (s_c, d_c, e_c) in cores:
        cnt = np.bincount(d_c, minlength=NPC).astype(np.int64)
        assert cnt.max() <= GEDGE
        grp = np.zeros(NPC, np.int64)
        slot = np.zeros(NPC, np.int64)
        order = np.argsort(-cnt, kind="stable")
        MAXOPEN = 64
        redges = np.zeros(0, np.int64)
        nslots = np.zeros(0, np.int64)
        gids = np.zeros(0, np.int64)
        ng = 0
        for n in order:
            cn = cnt[n]
            fits = (nslots < SLOTS) & (redges + cn <= GEDGE)
            j = int(np.argmax(fits)) if fits.any() else -1
            if j < 0:
                redges = np.concatenate([redges, [cn]])
                nslots = np.concatenate([nslots, [1]])
                gids = np.concatenate([gids, [ng]])
                grp[n] = ng
                slot[n] = 0
                ng += 1
                if len(gids) > MAXOPEN:
                    k = int(np.argmin(
                        (SLOTS - nslots) * GEDGE + (GEDGE - redges)))
                    redges = np.delete(redges, k)
                    nslots = np.delete(nslots, k)
                    gids = np.delete(gids, k)
            else:
                grp[n] = gids[j]
                slot[n] = nslots[j]
                redges[j] += cn
                nslots[j] += 1
        packed.append((s_c, d_c, e_c, grp, slot, ng))

    GREAL = max(p[-1] for p in packed)
    G = -(-GREAL // GPW) * GPW

    per_core = []
    for (s_c, d_c, e_c, grp, slot, _ng) in packed:
        ne = len(s_c)
        eg = grp[d_c]
        es = slot[d_c]
        o2 = np.lexsort((d_c, eg))
        eg2, es2 = eg[o2], es[o2]
        kb = np.flatnonzero(np.diff(eg2)) + 1
        kstarts = np.concatenate([[0], kb])
        r = np.arange(ne) - np.repeat(kstarts, np.diff(
            np.concatenate([kstarts, [ne]])))
        pos = eg2 * GEDGE + r                     # flat row in [G*512]
        NR = G * GEDGE

        esrc = np.zeros(NR, np.int64)
        esrc[pos] = s_c[o2]
        ea_rows = np.zeros((NR, D_EDGE), np.float32)
        ea_rows[pos] = e_c[o2]
        eslot = np.zeros(NR, np.int64)
        eslot[pos] = es2
        evalid = np.zeros(NR, np.float32)
        evalid[pos] = 1.0

        ea4 = ea_rows.reshape(G, NPH, EPT, D_EDGE)
        ev4 = evalid.reshape(G, NPH, EPT)
        rows = np.arange(NR)
        M4 = np.zeros((G, NPH, EPT, SLOTS), np.float32)
        M4[rows // GEDGE, (rows // EPT) % NPH, rows % EPT, eslot] = evalid

        # lhsT band stream [128, G, 2, 128]: band b=p%2 rows 64b..64b+64
        # hold phase p=2q+b at column-block q: rows +0:16 eaT, +16:48 Mt,
        # row +48 evalid (bl injector), rest zero
        ls4 = np.zeros((128, G, 2, EPT), np.float32)
        for p in range(NPH):
            b, q = p % 2, p // 2
            ls4[64 * b:64 * b + D_EDGE, :, q, :] = \
                ea4[:, p].transpose(2, 0, 1)
            ls4[64 * b + 16:64 * b + 16 + SLOTS, :, q, :] = \
                M4[:, p].transpose(2, 0, 1)
            ls4[64 * b + 48, :, q, :] = ev4[:, p]

        # M stream [128, G*4*SLOTS]
        Mflat = M4.transpose(2, 0, 1, 3).reshape(EPT, G * NPH * SLOTS)

        # slot -> node map
        slot_node = np.full(G * SLOTS, -1, np.int32)
        slot_node[grp * SLOTS + slot] = np.arange(NPC, dtype=np.int32)

        per_core.append(dict(
            ls=np.ascontiguousarray(
                ls4.reshape(128, G * 2 * EPT)).astype(BF16),
            M=np.ascontiguousarray(Mflat).astype(BF16),
            esrc=esrc, slot_node=slot_node))
    return per_core, G


def _build_layer(G, H, C, D_IN, do_elu):
    import concourse.bass as bass
    import concourse.mybir as mybir
    from concourse import bacc
    from concourse.tile import TileContext

    HC = H * C
    WP = HC + H
    S = G * SLOTS
    f32 = mybir.dt.float32
    bf16 = mybir.dt.bfloat16
    Alu = mybir.AluOpType
    Act = mybir.ActivationFunctionType
    NW = G // GPW

    nc = bacc.Bacc("TRN2", target_bir_lowering=False, debug=False,
                   num_devices=N_CORES)

    xeT_d = nc.dram_tensor("xeT", [D_IN, G * GEDGE], bf16,
                           kind="ExternalInput")
    xT_slots = nc.dram_tensor("xT_slots", [D_IN, G * 128], bf16,
                              kind="ExternalInput")
    wl = nc.dram_tensor("wl", [D_IN, HC], bf16, kind="ExternalInput")
    wr = nc.dram_tensor("wr", [D_IN, HC], bf16, kind="ExternalInput")
    webr = nc.dram_tensor("webr", [D_EDGE + 2, HC], bf16,
                          kind="ExternalInput")
    weB = nc.dram_tensor("weB", [D_EDGE + 2, 128], bf16,
                         kind="ExternalInput")
    attB = nc.dram_tensor("attB", [128, HC], bf16, kind="ExternalInput")
    biasC = nc.dram_tensor("biasC", [HC, 1], f32, kind="ExternalInput")
    nbiasC = nc.dram_tensor("nbiasC", [HC, 1], f32, kind="ExternalInput")
    onesB = nc.dram_tensor("onesB", [128, 1], bf16, kind="ExternalInput")
    exp8 = nc.dram_tensor("exp8", [H, HC], bf16, kind="ExternalInput")
    ls_d = nc.dram_tensor("ls", [128, G * 2 * EPT], bf16,
                          kind="ExternalInput")
    M_d = nc.dram_tensor("M", [128, G * NPH * SLOTS], bf16,
                         kind="ExternalInput")

    out_slots = nc.dram_tensor("out_slots", [HC, S], bf16,
                               kind="ExternalOutput")

    with TileContext(nc) as tc:
        with tc.tile_pool(name="const", bufs=1) as cpool:
            wl_t = cpool.tile([D_IN, HC], bf16)
            nc.sync.dma_start(wl_t[:], wl[:, :])
            wr_t = cpool.tile([D_IN, HC], bf16)
            nc.sync.dma_start(wr_t[:], wr[:, :])
            webr_t = cpool.tile([D_EDGE + 2, HC], bf16)
            nc.sync.dma_start(webr_t[:], webr[:, :])
            webr4_t = cpool.tile([D_EDGE + 2, 4, HC], bf16)
            wbv = webr_t[:, :]
            nc.vector.tensor_copy(
                out=webr4_t[:],
                in_=bass.AP(wbv.tensor, wbv.offset,
                            [wbv.ap[0], [0, 4], [1, HC]]))
            weB_t = cpool.tile([D_EDGE + 2, 128], bf16)
            nc.sync.dma_start(weB_t[:], weB[:, :])
            attB_t = cpool.tile([128, HC], bf16)
            nc.sync.dma_start(attB_t[:], attB[:, :])
            biasC_t = cpool.tile([HC, 1], f32)
            nc.sync.dma_start(biasC_t[:], biasC[:, :])
            nbiasC_t = cpool.tile([HC, 1], f32)
            nc.sync.dma_start(nbiasC_t[:], nbiasC[:, :])
            onesB_t = cpool.tile([128, 1], bf16)
            nc.sync.dma_start(onesB_t[:], onesB[:, :])
            exp8_t = cpool.tile([H, HC], bf16)
            nc.sync.dma_start(exp8_t[:], exp8[:, :])
            # att replicated GPW times for a flat contiguous zm multiply
            attW_t = cpool.tile([128, GPW * HC], bf16)
            ab0 = attB_t[:, :]
            nc.vector.tensor_copy(
                out=attW_t[:].rearrange("p (t c) -> p t c", c=HC),
                in_=bass.AP(ab0.tensor, ab0.offset,
                            [ab0.ap[0], [0, GPW], [1, HC]]))

            with tc.tile_pool(name="strm", bufs=2) as spool, \
                 tc.tile_pool(name="xe", bufs=2) as xpool, \
                 tc.tile_pool(name="rhs", bufs=2) as rpool, \
                 tc.tile_pool(name="work", bufs=2) as wpool, \
                 tc.tile_pool(name="bnc", bufs=2) as bpool, \
                 tc.tile_pool(name="zps", bufs=3, space="PSUM") as zps, \
                 tc.tile_pool(name="rps", bufs=2, space="PSUM") as rps, \
                 tc.tile_pool(name="xps", bufs=1, space="PSUM") as xps, \
                 tc.tile_pool(name="ops", bufs=2, space="PSUM") as ops:

                # static [We;br;bl] band content, built once:
                # rhs_all(window) = weconst + xr matmuls
                weconst = cpool.tile([128, 4, HC], bf16)
                prc = rps.tile([128, 4, HC], f32, space="PSUM", tag="pr")
                nc.tensor.matmul(
                    out=prc[:], lhsT=weB_t[:], rhs=webr4_t[:],
                    start=True, stop=True, skip_group_check=True)
                nc.vector.tensor_copy(out=weconst[:], in_=prc[:])

                def _emit_finalize(pso, w):
                    # transpose-free finalize in [hc-rows, slot-cols].
                    # +eps guards PAD slots (denominator exactly 0 there;
                    # approx_fast(0) is NaN and 0*NaN leaks via the mms)
                    s_eps = bpool.tile([H, 512], f32, tag="s")
                    nc.vector.tensor_scalar_add(
                        s_eps[:], pso[HC:HC + H, :], 1e-16)
                    rec = bpool.tile([H, 512], f32, tag="rec")
                    nc.vector.reciprocal_approx_fast(
                        out=rec[:], in_=s_eps[:])
                    rec_b = bpool.tile([H, 512], bf16, tag="recb")
                    nc.scalar.activation(rec_b[:], rec[:], Act.Copy)
                    recx_ps = xps.tile([HC, 512], f32, space="PSUM",
                                       tag="recx")
                    nc.tensor.matmul(
                        out=recx_ps[:], lhsT=exp8_t[:], rhs=rec_b[:],
                        start=True, stop=True, skip_group_check=True)
                    recx = bpool.tile([HC, 512], f32, tag="recxs")
                    nc.scalar.activation(recx[:], recx_ps[:], Act.Copy)
                    o = bpool.tile([HC, 512], f32, tag="o")
                    nc.vector.tensor_tensor(
                        out=o[:], in0=pso[0:HC, :], in1=recx[:],
                        op=Alu.mult)
                    ob = bpool.tile([HC, 512], bf16, tag="ob")
                    if do_elu:
                        # ELU(o+b) = relu(o+b) + exp(-relu(-(o+b))) - 1,
                        # biases applied via per-partition ACT bias APs
                        pos = bpool.tile([HC, 512], f32, tag="pos")
                        nc.scalar.activation(pos[:], o[:], Act.Relu,
                                             bias=biasC_t[:])
                        t1 = bpool.tile([HC, 512], f32, tag="t1")
                        nc.scalar.activation(t1[:], o[:], Act.Relu,
                                             scale=-1.0,
                                             bias=nbiasC_t[:])
                        en = bpool.tile([HC, 512], f32, tag="en")
                        nc.scalar.activation(en[:], t1[:], Act.Exp,
                                             scale=-1.0)
                        nc.vector.scalar_tensor_tensor(
                            out=ob[:], in0=en[:], scalar=-1.0,
                            in1=pos[:], op0=Alu.add, op1=Alu.add)
                    else:
                        nc.scalar.activation(ob[:], o[:], Act.Identity,
                                             bias=biasC_t[:])
                    nc.sync.dma_start(
                        out_slots[:, w * 512:(w + 1) * 512], ob[:])

                def _dma_xst(w):
                    xst = spool.tile([D_IN, GPW * 128], bf16, tag="xs")
                    nc.sync.dma_start(
                        xst[:], xT_slots[:, w * GPW * 128:
                                         (w + 1) * GPW * 128])
                    return xst

                def _emit_rhs_mms(xst):
                    # rhs_all [128, GPW, HC]: per group two 64-row bands
                    # rows +0:16 We, +16:48 xr slots, +48 bl, rest 0
                    rhs_all = rpool.tile([128, GPW, HC], bf16, tag="r",
                                         space="SBUF")
                    for g4 in range(GPW // 4):
                        pr = rps.tile([128, 4, HC], f32, space="PSUM",
                                      tag="pr")
                        for jj in range(4):
                            gi = g4 * 4 + jj
                            nc.tensor.matmul(
                                out=pr[:, jj, :],
                                lhsT=xst[:, gi * 128:(gi + 1) * 128],
                                rhs=wr_t[:], start=(jj == 0),
                                stop=(jj == 3),
                                skip_group_check=True)
                        nc.vector.tensor_tensor(
                            out=rhs_all[:, g4 * 4:(g4 + 1) * 4, :],
                            in0=pr[:], in1=weconst[:], op=Alu.add)
                    return rhs_all

                fin_prev = None
                rhs_cur = None
                xst_next = _dma_xst(0)
                for w in range(NW):
                    g0 = w * GPW
                    ls_t = spool.tile([128, GPW * 2 * EPT], bf16,
                                      tag="ls")
                    nc.sync.dma_start(
                        ls_t[:], ls_d[:, g0 * 2 * EPT:
                                      (g0 + GPW) * 2 * EPT])
                    M_t = spool.tile([128, GPW * NPH * SLOTS], bf16,
                                     tag="M")
                    nc.sync.dma_start(
                        M_t[:], M_d[:, g0 * NPH * SLOTS:
                                    (g0 + GPW) * NPH * SLOTS])
                    xe_t = xpool.tile([D_IN, GPW * GEDGE], bf16, tag="xe")
                    nc.scalar.dma_start(
                        xe_t[:], xeT_d[:, g0 * GEDGE:(g0 + GPW) * GEDGE])

                    if rhs_cur is None:
                        # first window: build inline
                        rhs_cur = _emit_rhs_mms(xst_next)
                    if w + 1 < NW:
                        xst_next = _dma_xst(w + 1)
                    rhs_all = rhs_cur

                    pso = ops.tile([WP, GPW * SLOTS], f32, space="PSUM",
                                   tag="pso")
                    if H == 1:
                        # H=1: p is folded into the scatter rhs (M*p), so
                        # the lhsT is just [xl | 1] — xl copied straight
                        # into wp slots, no separate xl*p multiply
                        mp_t = wpool.tile([128, GPW * NPH * SLOTS], bf16,
                                          tag="mp")
                    wps = []
                    for p in range(NPH):
                        b64 = 64 * (p % 2)
                        q = p // 2
                        wp_t = wpool.tile([128, GPW * WP], bf16,
                                          tag=f"wp{p}")
                        wpv = wp_t[:, :]
                        if H > 1:
                            xl_sb = wpool.tile([128, GPW * HC], bf16,
                                               tag=f"xl{p}")
                        z0 = wpool.tile([128, GPW * HC], bf16,
                                        tag=f"z0{p}")
                        pszs = []
                        for h in range(2):
                            psz = zps.tile([128, 8 * HC], f32,
                                           space="PSUM", tag="psz")
                            # exactly ONE start=True per psz tile (the
                            # first mm): start=True clears has_written
                            # bank-wide, so later slices must use
                            # start=False and rely on per-element
                            # has_written (write-if-clear, else add)
                            for j in range(8):
                                gi = h * 8 + j
                                nc.tensor.matmul(
                                    out=psz[:, j * HC:(j + 1) * HC],
                                    lhsT=xe_t[:, (gi * NPH + p) * EPT:
                                              (gi * NPH + p + 1) * EPT],
                                    rhs=wl_t[:], start=(j == 0),
                                    stop=False,
                                    skip_group_check=True)
                            if H == 1:
                                nc.scalar.activation(
                                    bass.AP(wpv.tensor,
                                            wpv.offset + h * 8 * WP,
                                            [wpv.ap[0], [WP, 8],
                                             [1, HC]]),
                                    psz[:], Act.Copy)
                            else:
                                nc.scalar.activation(
                                    xl_sb[:, h * 8 * HC:
                                          (h + 1) * 8 * HC],
                                    psz[:], Act.Copy)
                            pszs.append(psz)
                        for h in range(2):
                            psz = pszs[h]
                            for j in range(8):
                                gi = h * 8 + j
                                lcol = (gi * 2 + q) * EPT
                                nc.tensor.matmul(
                                    out=psz[:, j * HC:(j + 1) * HC],
                                    lhsT=ls_t[b64:b64 + 64,
                                              lcol:lcol + EPT],
                                    rhs=rhs_all[b64:b64 + 64, gi, :],
                                    start=False, stop=(j == 7),
                                    skip_group_check=True)
                            # z = LeakyReLU(s) on the scalar engine
                            nc.scalar.activation(
                                z0[:, h * 8 * HC:(h + 1) * 8 * HC],
                                psz[:], Act.Prelu, alpha=NEG_SLOPE)
                        zm = wpool.tile([128, GPW * HC], bf16,
                                        tag="zm")
                        nc.vector.tensor_tensor(
                            out=zm[:], in0=z0[:], in1=attW_t[:],
                            op=Alu.mult)
                        sc = wpool.tile([128, GPW * H], bf16,
                                        tag="sc")
                        with nc.allow_low_precision(
                                reason="bf16 score sum, |sc|~O(1)"):
                            if H > 1:
                                # columns are c-major (c,h): sum over c
                                # as a 3-step contiguous halving tree
                                # (full-rate DVE, no inner-8 penalty)
                                zv = zm[:, :]
                                t1 = wpool.tile([128, GPW * HC // 2],
                                                bf16, tag="t1r")
                                nc.vector.tensor_tensor(
                                    out=t1[:],
                                    in0=bass.AP(zv.tensor, zv.offset,
                                                [zv.ap[0], [HC, GPW],
                                                 [1, HC // 2]]),
                                    in1=bass.AP(zv.tensor,
                                                zv.offset + HC // 2,
                                                [zv.ap[0], [HC, GPW],
                                                 [1, HC // 2]]),
                                    op=Alu.add)
                                tv = t1[:, :]
                                t2 = wpool.tile([128, GPW * HC // 4],
                                                bf16, tag="t2r")
                                nc.vector.tensor_tensor(
                                    out=t2[:],
                                    in0=bass.AP(tv.tensor, tv.offset,
                                                [tv.ap[0], [HC // 2,
                                                            GPW],
                                                 [1, HC // 4]]),
                                    in1=bass.AP(tv.tensor,
                                                tv.offset + HC // 4,
                                                [tv.ap[0], [HC // 2,
                                                            GPW],
                                                 [1, HC // 4]]),
                                    op=Alu.add)
                                uv = t2[:, :]
                                nc.vector.tensor_tensor(
                                    out=sc[:],
                                    in0=bass.AP(uv.tensor, uv.offset,
                                                [uv.ap[0], [HC // 4,
                                                            GPW],
                                                 [1, HC // 8]]),
                                    in1=bass.AP(uv.tensor,
                                                uv.offset + HC // 8,
                                                [uv.ap[0], [HC // 4,
                                                            GPW],
                                                 [1, HC // 8]]),
                                    op=Alu.add)
                            else:
                                # H=1: two contiguous halvings, then a
                                # short 16-wide reduce
                                zv = zm[:, :]
                                t1 = wpool.tile([128, GPW * C // 2],
                                                bf16, tag="t1r")
                                nc.vector.tensor_tensor(
                                    out=t1[:],
                                    in0=bass.AP(zv.tensor, zv.offset,
                                                [zv.ap[0], [C, GPW],
                                                 [1, C // 2]]),
                                    in1=bass.AP(zv.tensor,
                                                zv.offset + C // 2,
                                                [zv.ap[0], [C, GPW],
                                                 [1, C // 2]]),
                                    op=Alu.add)
                                tv = t1[:, :]
                                t2 = wpool.tile([128, GPW * C // 4],
                                                bf16, tag="t2r")
                                nc.vector.tensor_tensor(
                                    out=t2[:],
                                    in0=bass.AP(tv.tensor, tv.offset,
                                                [tv.ap[0], [C // 2,
                                                            GPW],
                                                 [1, C // 4]]),
                                    in1=bass.AP(tv.tensor,
                                                tv.offset + C // 4,
                                                [tv.ap[0], [C // 2,
                                                            GPW],
                                                 [1, C // 4]]),
                                    op=Alu.add)
                                nc.vector.tensor_reduce(
                                    out=sc[:],
                                    in_=t2[:].rearrange(
                                        "p (t k) -> p t k", k=C // 4),
                                    axis=mybir.AxisListType.X,
                                    op=Alu.add)
                        if H == 1:
                            pv = wpool.tile([128, GPW], bf16,
                                            tag=f"pv{p}")
                            nc.scalar.activation(pv[:], sc[:], Act.Exp)
                            # lhsT ones column (denominator row of pso)
                            ov = onesB_t[:, :]
                            nc.vector.tensor_copy(
                                out=bass.AP(wpv.tensor, wpv.offset + HC,
                                            [wpv.ap[0], [WP, GPW],
                                             [1, 1]]),
                                in_=bass.AP(ov.tensor, ov.offset,
                                            [ov.ap[0], [0, GPW],
                                             [1, 1]]))
                            # scatter rhs = M * p (per-edge row scale)
                            mpv = mp_t[:, :]
                            Mtv = M_t[:, :]
                            pvv = pv[:, :]
                            nc.vector.tensor_tensor(
                                out=bass.AP(mpv.tensor,
                                            mpv.offset + p * SLOTS,
                                            [mpv.ap[0],
                                             [NPH * SLOTS, GPW],
                                             [1, SLOTS]]),
                                in0=bass.AP(Mtv.tensor,
                                            Mtv.offset + p * SLOTS,
                                            [Mtv.ap[0],
                                             [NPH * SLOTS, GPW],
                                             [1, SLOTS]]),
                                in1=bass.AP(pvv.tensor, pvv.offset,
                                            [pvv.ap[0], [1, GPW],
                                             [0, SLOTS]]),
                                op=Alu.mult)
                        else:
                            p_out = bass.AP(wpv.tensor, wpv.offset + HC,
                                            [wpv.ap[0], [WP, GPW],
                                             [1, H]])
                            nc.scalar.activation(p_out, sc[:], Act.Exp)
                            # w = xl * exp(sc); columns are c-major, so
                            # the p broadcast reads contiguous H-runs
                            w_out = bass.AP(wpv.tensor, wpv.offset,
                                            [wpv.ap[0], [WP, GPW],
                                             [H, C], [1, H]])
                            pe_b = bass.AP(wpv.tensor, wpv.offset + HC,
                                           [wpv.ap[0], [WP, GPW],
                                            [0, C], [1, H]])
                            nc.vector.tensor_tensor(
                                out=w_out,
                                in0=xl_sb[:].rearrange(
                                    "p (t c h) -> p t c h", c=C, h=H),
                                in1=pe_b, op=Alu.mult)
                        wps.append(wp_t)
                        if p == 1 and w + 1 < NW:
                            # build next window's rhs mid-window: its
                            # xst DMA (issued at window top) has landed,
                            # and phase-0 bands of w+1 won't stall on it
                            rhs_next = _emit_rhs_mms(xst_next)
                    # per slice: the 4 phase mms CONSECUTIVELY
                    # (start=True clears has_written bank-wide)
                    sc_rhs = mp_t if H == 1 else M_t
                    for j in range(GPW):
                        for p in range(NPH):
                            nc.tensor.matmul(
                                out=pso[:, j * SLOTS:(j + 1) * SLOTS],
                                lhsT=wps[p][:, j * WP:(j + 1) * WP],
                                rhs=sc_rhs[:, (j * NPH + p) * SLOTS:
                                           (j * NPH + p + 1) * SLOTS],
                                start=(p == 0), stop=(p == 3),
                                skip_group_check=True)

                    # finalize is emitted one window late (software
                    # pipeline): its cross-engine chain then never
                    # head-of-line-blocks the engine queues, since all
                    # its inputs were produced a full window earlier
                    if fin_prev is not None:
                        _emit_finalize(*fin_prev)
                    fin_prev = (pso, w)
                    if w + 1 < NW:
                        rhs_cur = rhs_next
                _emit_finalize(*fin_prev)

    nc.compile()
    return nc


def _run(nc, in_maps, trace=False):
    from concourse.bass_utils import run_bass_kernel_spmd
    return run_bass_kernel_spmd(nc, in_maps, core_ids=list(range(N_CORES)),
                                trace=trace)


def kernel(x, edge_index, edge_attr,
           Wl1, bl1, Wr1, br1, We1, att1, b1,
           Wl2, bl2, Wr2, br2, We2, att2, b2,
           _trace=False, _times=None):
    x = np.asarray(x, np.float32)
    per_core, G = _preprocess(np.asarray(edge_index),
                              np.asarray(edge_attr))
    S = G * SLOTS

    def bcast(v):
        v = np.asarray(v, np.float32).reshape(-1)
        return np.broadcast_to(v[None, :], (128, v.shape[0])).astype(BF16)

    def layer_inputs(xf, Wl, bl, Wr, br, We, att, b, D_IN, HC, H):
        C = HC // H

        def cmaj(a):
            # reorder feature columns (h,c) -> (c,h): the on-device score
            # reduce over c then runs on contiguous column halves
            a = np.asarray(a, np.float32)
            return a.reshape(*a.shape[:-1], H, C).swapaxes(-1, -2).reshape(
                *a.shape[:-1], HC)

        # weB [18, 128]: We/br/bl injector lhsT for the rhs_all build
        weB = np.zeros((D_EDGE + 2, 128), np.float32)
        weB[np.arange(D_EDGE), np.arange(D_EDGE)] = 1.0
        weB[np.arange(D_EDGE), 64 + np.arange(D_EDGE)] = 1.0
        weB[D_EDGE, 16:48] = 1.0
        weB[D_EDGE, 80:112] = 1.0
        weB[D_EDGE + 1, 48] = 1.0
        weB[D_EDGE + 1, 112] = 1.0
        webr = cmaj(np.concatenate(
            [np.asarray(We, np.float32),
             np.asarray(br, np.float32)[None, :],
             np.asarray(bl, np.float32)[None, :]], axis=0))
        # output bias absorbs bl (sum of alpha over a segment is 1)
        bout = cmaj(np.asarray(b, np.float32).reshape(-1)
                    + np.asarray(bl, np.float32).reshape(-1))
        att_f = cmaj(np.asarray(att, np.float32).reshape(-1))
        Wl = cmaj(Wl)
        Wr = cmaj(Wr)
        e8 = np.zeros((H, HC), np.float32)
        e8[np.arange(HC) % H, np.arange(HC)] = 1.0
        maps = []
        for c in range(N_CORES):
            pc = per_core[c]
            sn = pc["slot_node"]
            valid = sn >= 0
            # per-edge source features, transposed: [D_IN, G*512]
            xeT = np.ascontiguousarray(
                xf[pc["esrc"]].T).astype(BF16)
            # xT_slots [D_IN, G*128]: per group cols 16..48 and 80..112
            # hold the group's 32 slot features (two replicas), rest zero
            xs = np.zeros((G, 128, xf.shape[1]), np.float32)
            feats = np.zeros((G * SLOTS, xf.shape[1]), np.float32)
            feats[valid] = xf[sn[valid].astype(np.int64) + c * NPC]
            fg = feats.reshape(G, SLOTS, -1)
            xs[:, 16:48, :] = fg
            xs[:, 80:112, :] = fg
            xsT = np.ascontiguousarray(
                xs.reshape(G * 128, -1).T).astype(BF16)
            maps.append(dict(
                xeT=xeT, xT_slots=xsT,
                wl=np.asarray(Wl, np.float32).astype(BF16),
                wr=np.asarray(Wr, np.float32).astype(BF16),
                webr=webr.astype(BF16), weB=weB.astype(BF16),
                attB=bcast(att_f),
                biasC=bout.reshape(-1, 1).astype(np.float32),
                nbiasC=(-bout).reshape(-1, 1).astype(np.float32),
                onesB=np.ones((128, 1), BF16),
                exp8=e8.astype(BF16),
                ls=pc["ls"], M=pc["M"]))
        return maps

    def collect(res, width, H):
        # device rows are (c,h) c-major; un-permute back to (h,c)
        C = width // H
        out = np.zeros((N_NODES, width), np.float32)
        for c in range(N_CORES):
            sn = per_core[c]["slot_node"]
            valid = sn >= 0
            arr = np.asarray(res.results[c]["out_slots"]).astype(
                np.float32).T
            arr = arr.reshape(-1, C, H).swapaxes(1, 2).reshape(-1, width)
            out[sn[valid].astype(np.int64) + c * NPC] = arr[valid]
        return out

    nc1 = _build_layer(G, H1, C1, D_NODE, do_elu=True)
    res1 = _run(nc1, layer_inputs(x, Wl1, bl1, Wr1, br1, We1, att1, b1,
                                  D_NODE, H1 * C1, H1), trace=_trace)
    h = collect(res1, H1 * C1, H1)

    nc2 = _build_layer(G, 1, D_EMB, H1 * C1, do_elu=False)
    res2 = _run(nc2, layer_inputs(h, Wl2, bl2, Wr2, br2, We2, att2, b2,
                                  H1 * C1, D_EMB, 1), trace=_trace)
    out = collect(res2, D_EMB, 1)
    if _times is not None:
        _times.extend([res1.exec_time_ns, res2.exec_time_ns])
    return out


# revision 43
# speedup vs baseline: 1.1438x; 1.1438x over previous
"""GATv2 (2-layer, PyG-style self-loops) on 8 Trainium2 NeuronCores — bf16.

No dma_gather: the host stages per-edge source features x[src] in edge
order (pure layout), and the device projects them per-edge (lhsT=xeT
tile, rhs=Wl) straight into the score PSUM. This removes the SWDGE Q7
descriptor-generation serial bottleneck (~1ms/layer) and the table-build
prologue of the gather-based design.

Sharding: dst nodes split across 8 cores (12500 each); edges routed to the
core owning dst. Nodes packed into SLOT-GROUPS of <=32 slots and <=512
edges; each group's edges fill 4 tiles of 128 rows ("phases" p=row//128).
All feature columns are stored c-major (c,h) so the score reduce over c
runs as contiguous column halvings on the DVE.

Per phase-tile (bf16, PSUM fp32):
  psz  - 8 proj matmuls (lhsT=xeT 128-col tile, rhs=Wl) write xl per edge
         into PSUM (one start=True per tile; start clears has_written
         bank-wide), a scalar-engine Copy extracts xl to SBUF (for the
         message; H=1 writes straight into the wp slots), then 8 band
         matmuls accumulate ee+xr+biases: lhsT=[eaT(16);Mt(32);evalid]
         band, rhs=[We;br;bl;xr_g] (weconst + per-group xr matmuls).
  z    - LeakyReLU on the scalar engine.
  p    - z*att (DVE), halving-tree reduce, Exp on scalar engine.
  out  - lhsT=[xl*p | p] (H=1: [xl | 1] with p folded into the scatter
         rhs M*p) contracted with the one-hot M into a 512-slot PSUM
         window, 4 phases per slice consecutively.
Finalize (emitted one window late so its cross-engine chain never blocks
queue heads): reciprocal_approx_fast of the denominators (+eps for pad
slots), matmul-expanded to [HC,512], normalize, bias (+bl since
sum-alpha=1) and ELU via scalar-engine Relu/Exp with bias APs; output
stays [HC, S] (host transposes).
"""

import numpy as np
import ml_dtypes

BF16 = ml_dtypes.bfloat16

N_NODES = 100000
D_EDGE = 16
H1, C1 = 8, 8
D_NODE = 128
D_EMB = 64
NEG_SLOPE = 0.2
N_CORES = 8
NPC = N_NODES // N_CORES          # 12500 dst nodes per core
SLOTS = 32                        # slots per group
EPT = 128                         # edge rows per phase-tile
NPH = 4                           # tiles (phases) per group
GEDGE = NPH * EPT                 # 512 edge rows per group
GPW = 16                          # groups per psum window (512 slots)


def _preprocess(edge_index, edge_attr):
    src = np.asarray(edge_index[0], dtype=np.int64)
    dst = np.asarray(edge_index[1], dtype=np.int64)
    ea = np.asarray(edge_attr, dtype=np.float32)

    deg = np.bincount(dst, minlength=N_NODES).astype(np.float32)
    order0 = np.argsort(dst, kind="stable")
    ds = dst[order0]
    bnd0 = np.flatnonzero(np.diff(ds)) + 1
    starts0 = np.concatenate([[0], bnd0])
    ea_sum = np.zeros((N_NODES, D_EDGE), np.float32)
    ea_sum[ds[starts0]] = np.add.reduceat(ea[order0], starts0, axis=0)
    ea_mean = ea_sum / np.maximum(deg, 1.0)[:, None]

    loop = np.arange(N_NODES, dtype=np.int64)
    src2 = np.concatenate([src, loop])
    dst2 = np.concatenate([dst, loop])
    ea2 = np.concatenate([ea, ea_mean], axis=0)

    cores = []
    for c in range(N_CORES):
        lo = c * NPC
        m = (dst2 >= lo) & (dst2 < lo + NPC)
        cores.append((src2[m], dst2[m] - lo, ea2[m]))

    # --- per-core grouping: <=32 slots/group, <=512 edges/group (FFD) ---
    packed = []
    for (s_c, d_c, e_c) in cores:
        cnt = np.bincount(d_c, minlength=NPC).astype(np.int64)
        assert cnt.max() <= GEDGE
        grp = np.zeros(NPC, np.int64)
        slot = np.zeros(NPC, np.int64)
        order = np.argsort(-cnt, kind="stable")
        MAXOPEN = 64
        redges = np.zeros(0, np.int64)
        nslots = np.zeros(0, np.int64)
        gids = np.zeros(0, np.int64)
        ng = 0
        for n in order:
            cn = cnt[n]
            fits = (nslots < SLOTS) & (redges + cn <= GEDGE)
            j = int(np.argmax(fits)) if fits.any() else -1
            if j < 0:
                redges = np.concatenate([redges, [cn]])
                nslots = np.concatenate([nslots, [1]])
                gids = np.concatenate([gids, [ng]])
                grp[n] = ng
                slot[n] = 0
                ng += 1
                if len(gids) > MAXOPEN:
                    k = int(np.argmin(
                        (SLOTS - nslots) * GEDGE + (GEDGE - redges)))
                    redges = np.delete(redges, k)
                    nslots = np.delete(nslots, k)
                    gids = np.delete(gids, k)
            else:
                grp[n] = gids[j]
                slot[n] = nslots[j]
                redges[j] += cn
                nslots[j] += 1
        packed.append((s_c, d_c, e_c, grp, slot, ng))

    GREAL = max(p[-1] for p in packed)
    G = -(-GREAL // GPW) * GPW

    per_core = []
    for (s_c, d_c, e_c, grp, slot, _ng) in packed:
        ne = len(s_c)
        eg = grp[d_c]
        es = slot[d_c]
        o2 = np.lexsort((d_c, eg))
        eg2, es2 = eg[o2], es[o2]
        kb = np.flatnonzero(np.diff(eg2)) + 1
        kstarts = np.concatenate([[0], kb])
        r = np.arange(ne) - np.repeat(kstarts, np.diff(
            np.concatenate([kstarts, [ne]])))
        pos = eg2 * GEDGE + r                     # flat row in [G*512]
        NR = G * GEDGE

        esrc = np.zeros(NR, np.int64)
        esrc[pos] = s_c[o2]
        ea_rows = np.zeros((NR, D_EDGE), np.float32)
        ea_rows[pos] = e_c[o2]
        eslot = np.zeros(NR, np.int64)
        eslot[pos] = es2
        evalid = np.zeros(NR, np.float32)
        evalid[pos] = 1.0

        ea4 = ea_rows.reshape(G, NPH, EPT, D_EDGE)
        ev4 = evalid.reshape(G, NPH, EPT)
        rows = np.arange(NR)
        M4 = np.zeros((G, NPH, EPT, SLOTS), np.float32)
        M4[rows // GEDGE, (rows // EPT) % NPH, rows % EPT, eslot] = evalid

        # lhsT band stream [128, G, 2, 128]: band b=p%2 rows 64b..64b+64
        # hold phase p=2q+b at column-block q: rows +0:16 eaT, +16:48 Mt,
        # row +48 evalid (bl injector), rest zero
        ls4 = np.zeros((128, G, 2, EPT), np.float32)
        for p in range(NPH):
            b, q = p % 2, p // 2
            ls4[64 * b:64 * b + D_EDGE, :, q, :] = \
                ea4[:, p].transpose(2, 0, 1)
            ls4[64 * b + 16:64 * b + 16 + SLOTS, :, q, :] = \
                M4[:, p].transpose(2, 0, 1)
            ls4[64 * b + 48, :, q, :] = ev4[:, p]

        # M stream [128, G*4*SLOTS]
        Mflat = M4.transpose(2, 0, 1, 3).reshape(EPT, G * NPH * SLOTS)

        # slot -> node map
        slot_node = np.full(G * SLOTS, -1, np.int32)
        slot_node[grp * SLOTS + slot] = np.arange(NPC, dtype=np.int32)

        per_core.append(dict(
            ls=np.ascontiguousarray(
                ls4.reshape(128, G * 2 * EPT)).astype(BF16),
            M=np.ascontiguousarray(Mflat).astype(BF16),
            esrc=esrc, slot_node=slot_node))
    return per_core, G


def _build_layer(G, H, C, D_IN, do_elu):
    import concourse.bass as bass
    import concourse.mybir as mybir
    from concourse import bacc
    from concourse.tile import TileContext

    HC = H * C
    WP = HC + H
    S = G * SLOTS
    f32 = mybir.dt.float32
    bf16 = mybir.dt.bfloat16
    Alu = mybir.AluOpType
    Act = mybir.ActivationFunctionType
    NW = G // GPW

    nc = bacc.Bacc("TRN2", target_bir_lowering=False, debug=False,
                   num_devices=N_CORES)

    xeT_d = nc.dram_tensor("xeT", [D_IN, G * GEDGE], bf16,
                           kind="ExternalInput")
    xT_slots = nc.dram_tensor("xT_slots", [D_IN, G * 128], bf16,
                              kind="ExternalInput")
    wl = nc.dram_tensor("wl", [D_IN, HC], bf16, kind="ExternalInput")
    wr = nc.dram_tensor("wr", [D_IN, HC], bf16, kind="ExternalInput")
    webr = nc.dram_tensor("webr", [D_EDGE + 2, HC], bf16,
                          kind="ExternalInput")
    weB = nc.dram_tensor("weB", [D_EDGE + 2, 128], bf16,
                         kind="ExternalInput")
    attB = nc.dram_tensor("attB", [128, HC], bf16, kind="ExternalInput")
    biasC = nc.dram_tensor("biasC", [HC, 1], f32, kind="ExternalInput")
    nbiasC = nc.dram_tensor("nbiasC", [HC, 1], f32, kind="ExternalInput")
    onesB = nc.dram_tensor("onesB", [128, 1], bf16, kind="ExternalInput")
    exp8 = nc.dram_tensor("exp8", [H, HC], bf16, kind="ExternalInput")
    ls_d = nc.dram_tensor("ls", [128, G * 2 * EPT], bf16,
                          kind="ExternalInput")
    M_d = nc.dram_tensor("M", [128, G * NPH * SLOTS], bf16,
                         kind="ExternalInput")

    out_slots = nc.dram_tensor("out_slots", [HC, S], bf16,
                               kind="ExternalOutput")

    with TileContext(nc) as tc:
        with tc.tile_pool(name="const", bufs=1) as cpool:
            wl_t = cpool.tile([D_IN, HC], bf16)
            nc.sync.dma_start(wl_t[:], wl[:, :])
            wr_t = cpool.tile([D_IN, HC], bf16)
            nc.sync.dma_start(wr_t[:], wr[:, :])
            webr_t = cpool.tile([D_EDGE + 2, HC], bf16)
            nc.sync.dma_start(webr_t[:], webr[:, :])
            webr4_t = cpool.tile([D_EDGE + 2, 4, HC], bf16)
            wbv = webr_t[:, :]
            nc.vector.tensor_copy(
                out=webr4_t[:],
                in_=bass.AP(wbv.tensor, wbv.offset,
                            [wbv.ap[0], [0, 4], [1, HC]]))
            weB_t = cpool.tile([D_EDGE + 2, 128], bf16)
            nc.sync.dma_start(weB_t[:], weB[:, :])
            attB_t = cpool.tile([128, HC], bf16)
            nc.sync.dma_start(attB_t[:], attB[:, :])
            biasC_t = cpool.tile([HC, 1], f32)
            nc.sync.dma_start(biasC_t[:], biasC[:, :])
            nbiasC_t = cpool.tile([HC, 1], f32)
            nc.sync.dma_start(nbiasC_t[:], nbiasC[:, :])
            onesB_t = cpool.tile([128, 1], bf16)
            nc.sync.dma_start(onesB_t[:], onesB[:, :])
            exp8_t = cpool.tile([H, HC], bf16)
            nc.sync.dma_start(exp8_t[:], exp8[:, :])
            # att replicated GPW times for a flat contiguous zm multiply
            attW_t = cpool.tile([128, GPW * HC], bf16)
            ab0 = attB_t[:, :]
            nc.vector.tensor_copy(
                out=attW_t[:].rearrange("p (t c) -> p t c", c=HC),
                in_=bass.AP(ab0.tensor, ab0.offset,
                            [ab0.ap[0], [0, GPW], [1, HC]]))

            with tc.tile_pool(name="strm", bufs=2) as spool, \
                 tc.tile_pool(name="xe", bufs=2) as xpool, \
                 tc.tile_pool(name="rhs", bufs=2) as rpool, \
                 tc.tile_pool(name="work", bufs=2) as wpool, \
                 tc.tile_pool(name="bnc", bufs=2) as bpool, \
                 tc.tile_pool(name="zps", bufs=3, space="PSUM") as zps, \
                 tc.tile_pool(name="rps", bufs=2, space="PSUM") as rps, \
                 tc.tile_pool(name="xps", bufs=1, space="PSUM") as xps, \
                 tc.tile_pool(name="ops", bufs=2, space="PSUM") as ops:

                # static [We;br;bl] band content, built once:
                # rhs_all(window) = weconst + xr matmuls
                weconst = cpool.tile([128, 4, HC], bf16)
                prc = rps.tile([128, 4, HC], f32, space="PSUM", tag="pr")
                nc.tensor.matmul(
                    out=prc[:], lhsT=weB_t[:], rhs=webr4_t[:],
                    start=True, stop=True, skip_group_check=True)
                nc.vector.tensor_copy(out=weconst[:], in_=prc[:])

                def _emit_finalize(pso, w):
                    # transpose-free finalize in [hc-rows, slot-cols].
                    # +eps guards PAD slots (denominator exactly 0 there;
                    # approx_fast(0) is NaN and 0*NaN leaks via the mms)
                    s_eps = bpool.tile([H, 512], f32, tag="s")
                    nc.vector.tensor_scalar_add(
                        s_eps[:], pso[HC:HC + H, :], 1e-16)
                    rec = bpool.tile([H, 512], f32, tag="rec")
                    nc.vector.reciprocal_approx_fast(
                        out=rec[:], in_=s_eps[:])
                    rec_b = bpool.tile([H, 512], bf16, tag="recb")
                    nc.scalar.activation(rec_b[:], rec[:], Act.Copy)
                    recx_ps = xps.tile([HC, 512], f32, space="PSUM",
                                       tag="recx")
                    nc.tensor.matmul(
                        out=recx_ps[:], lhsT=exp8_t[:], rhs=rec_b[:],
                        start=True, stop=True, skip_group_check=True)
                    recx = bpool.tile([HC, 512], f32, tag="recxs")
                    nc.scalar.activation(recx[:], recx_ps[:], Act.Copy)
                    o = bpool.tile([HC, 512], f32, tag="o")
                    nc.vector.tensor_tensor(
                        out=o[:], in0=pso[0:HC, :], in1=recx[:],
                        op=Alu.mult)
                    ob = bpool.tile([HC, 512], bf16, tag="ob")
                    if do_elu:
                        # ELU(o+b) = relu(o+b) + exp(-relu(-(o+b))) - 1,
                        # biases applied via per-partition ACT bias APs
                        pos = bpool.tile([HC, 512], f32, tag="pos")
                        nc.scalar.activation(pos[:], o[:], Act.Relu,
                                             bias=biasC_t[:])
                        t1 = bpool.tile([HC, 512], f32, tag="t1")
                        nc.scalar.activation(t1[:], o[:], Act.Relu,
                                             scale=-1.0,
                                             bias=nbiasC_t[:])
                        en = bpool.tile([HC, 512], f32, tag="en")
                        nc.scalar.activation(en[:], t1[:], Act.Exp,
                                             scale=-1.0)
                        nc.vector.scalar_tensor_tensor(
                            out=ob[:], in0=en[:], scalar=-1.0,
                            in1=pos[:], op0=Alu.add, op1=Alu.add)
                    else:
                        nc.scalar.activation(ob[:], o[:], Act.Identity,
                                             bias=biasC_t[:])
                    nc.sync.dma_start(
                        out_slots[:, w * 512:(w + 1) * 512], ob[:])

                fin_prev = None
                for w in range(NW):
                    g0 = w * GPW
                    ls_t = spool.tile([128, GPW * 2 * EPT], bf16,
                                      tag="ls")
                    nc.sync.dma_start(
                        ls_t[:], ls_d[:, g0 * 2 * EPT:
                                      (g0 + GPW) * 2 * EPT])
                    M_t = spool.tile([128, GPW * NPH * SLOTS], bf16,
                                     tag="M")
                    nc.sync.dma_start(
                        M_t[:], M_d[:, g0 * NPH * SLOTS:
                                    (g0 + GPW) * NPH * SLOTS])
                    xe_t = xpool.tile([D_IN, GPW * GEDGE], bf16, tag="xe")
                    nc.scalar.dma_start(
                        xe_t[:], xeT_d[:, g0 * GEDGE:(g0 + GPW) * GEDGE])

                    xst = spool.tile([D_IN, GPW * 128], bf16, tag="xs")
                    nc.sync.dma_start(
                        xst[:], xT_slots[:, g0 * 128:(g0 + GPW) * 128])
                    rhs_all = None

                    pso = ops.tile([WP, GPW * SLOTS], f32, space="PSUM",
                                   tag="pso")
                    if H == 1:
                        # H=1: p is folded into the scatter rhs (M*p), so
                        # the lhsT is just [xl | 1] — xl copied straight
                        # into wp slots, no separate xl*p multiply
                        mp_t = wpool.tile([128, GPW * NPH * SLOTS], bf16,
                                          tag="mp")
                    wps = []
                    for p in range(NPH):
                        b64 = 64 * (p % 2)
                        q = p // 2
                        wp_t = wpool.tile([128, GPW * WP], bf16,
                                          tag=f"wp{p}")
                        wpv = wp_t[:, :]
                        if H > 1:
                            xl_sb = wpool.tile([128, GPW * HC], bf16,
                                               tag=f"xl{p}")
                        z0 = wpool.tile([128, GPW * HC], bf16,
                                        tag=f"z0{p}")
                        pszs = []
                        for h in range(2):
                            psz = zps.tile([128, 8 * HC], f32,
                                           space="PSUM", tag="psz")
                            # exactly ONE start=True per psz tile (the
                            # first mm): start=True clears has_written
                            # bank-wide, so later slices must use
                            # start=False and rely on per-element
                            # has_written (write-if-clear, else add)
                            for j in range(8):
                                gi = h * 8 + j
                                nc.tensor.matmul(
                                    out=psz[:, j * HC:(j + 1) * HC],
                                    lhsT=xe_t[:, (gi * NPH + p) * EPT:
                                              (gi * NPH + p + 1) * EPT],
                                    rhs=wl_t[:], start=(j == 0),
                                    stop=False,
                                    skip_group_check=True)
                            if H == 1:
                                nc.scalar.activation(
                                    bass.AP(wpv.tensor,
                                            wpv.offset + h * 8 * WP,
                                            [wpv.ap[0], [WP, 8],
                                             [1, HC]]),
                                    psz[:], Act.Copy)
                            else:
                                nc.scalar.activation(
                                    xl_sb[:, h * 8 * HC:
                                          (h + 1) * 8 * HC],
                                    psz[:], Act.Copy)
                            pszs.append(psz)
                        if p == 0:
                            # rhs_all [128, GPW, HC]: per group two
                            # 64-row bands: +0:16 We, +16:48 xr slots,
                            # +48 bl. Built here (after phase-0 proj)
                            # so its DMA/add latency never FIFO-blocks
                            # independent proj work on the PE
                            rhs_all = rpool.tile([128, GPW, HC], bf16,
                                                 tag="r", space="SBUF")
                            for g4 in range(GPW // 4):
                                pr = rps.tile([128, 4, HC], f32,
                                              space="PSUM", tag="pr")
                                for jj in range(4):
                                    gi = g4 * 4 + jj
                                    nc.tensor.matmul(
                                        out=pr[:, jj, :],
                                        lhsT=xst[:, gi * 128:
                                                 (gi + 1) * 128],
                                        rhs=wr_t[:], start=(jj == 0),
                                        stop=(jj == 3),
                                        skip_group_check=True)
                                nc.vector.tensor_tensor(
                                    out=rhs_all[:, g4 * 4:
                                                (g4 + 1) * 4, :],
                                    in0=pr[:], in1=weconst[:],
                                    op=Alu.add)
                        for h in range(2):
                            psz = pszs[h]
                            for j in range(8):
                                gi = h * 8 + j
                                lcol = (gi * 2 + q) * EPT
                                nc.tensor.matmul(
                                    out=psz[:, j * HC:(j + 1) * HC],
                                    lhsT=ls_t[b64:b64 + 64,
                                              lcol:lcol + EPT],
                                    rhs=rhs_all[b64:b64 + 64, gi, :],
                                    start=False, stop=(j == 7),
                                    skip_group_check=True)
                            # z = LeakyReLU(s) on the scalar engine
                            nc.scalar.activation(
                                z0[:, h * 8 * HC:(h + 1) * 8 * HC],
                                psz[:], Act.Prelu, alpha=NEG_SLOPE)
                        zm = wpool.tile([128, GPW * HC], bf16,
                                        tag="zm")
                        nc.vector.tensor_tensor(
                            out=zm[:], in0=z0[:], in1=attW_t[:],
                            op=Alu.mult)
                        sc = wpool.tile([128, GPW * H], bf16,
                                        tag="sc")
                        with nc.allow_low_precision(
                                reason="bf16 score sum, |sc|~O(1)"):
                            if H > 1:
                                # columns are c-major (c,h): sum over c
                                # as a 3-step contiguous halving tree
                                # (full-rate DVE, no inner-8 penalty)
                                zv = zm[:, :]
                                t1 = wpool.tile([128, GPW * HC // 2],
                                                bf16, tag="t1r")
                                nc.vector.tensor_tensor(
                                    out=t1[:],
                                    in0=bass.AP(zv.tensor, zv.offset,
                                                [zv.ap[0], [HC, GPW],
                                                 [1, HC // 2]]),
                                    in1=bass.AP(zv.tensor,
                                                zv.offset + HC // 2,
                                                [zv.ap[0], [HC, GPW],
                                                 [1, HC // 2]]),
                                    op=Alu.add)
                                tv = t1[:, :]
                                t2 = wpool.tile([128, GPW * HC // 4],
                                                bf16, tag="t2r")
                                nc.vector.tensor_tensor(
                                    out=t2[:],
                                    in0=bass.AP(tv.tensor, tv.offset,
                                                [tv.ap[0], [HC // 2,
                                                            GPW],
                                                 [1, HC // 4]]),
                                    in1=bass.AP(tv.tensor,
                                                tv.offset + HC // 4,
                                                [tv.ap[0], [HC // 2,
                                                            GPW],
                                                 [1, HC // 4]]),
                                    op=Alu.add)
                                uv = t2[:, :]
                                nc.vector.tensor_tensor(
                                    out=sc[:],
                                    in0=bass.AP(uv.tensor, uv.offset,
                                                [uv.ap[0], [HC // 4,
                                                            GPW],
                                                 [1, HC // 8]]),
                                    in1=bass.AP(uv.tensor,
                                                uv.offset + HC // 8,
                                                [uv.ap[0], [HC // 4,
                                                            GPW],
                                                 [1, HC // 8]]),
                                    op=Alu.add)
                            else:
                                # H=1: two contiguous halvings, then a
                                # short 16-wide reduce
                                zv = zm[:, :]
                                t1 = wpool.tile([128, GPW * C // 2],
                                                bf16, tag="t1r")
                                nc.vector.tensor_tensor(
                                    out=t1[:],
                                    in0=bass.AP(zv.tensor, zv.offset,
                                                [zv.ap[0], [C, GPW],
                                                 [1, C // 2]]),
                                    in1=bass.AP(zv.tensor,
                                                zv.offset + C // 2,
                                                [zv.ap[0], [C, GPW],
                                                 [1, C // 2]]),
                                    op=Alu.add)
                                tv = t1[:, :]
                                t2 = wpool.tile([128, GPW * C // 4],
                                                bf16, tag="t2r")
                                nc.vector.tensor_tensor(
                                    out=t2[:],
                                    in0=bass.AP(tv.tensor, tv.offset,
                                                [tv.ap[0], [C // 2,
                                                            GPW],
                                                 [1, C // 4]]),
                                    in1=bass.AP(tv.tensor,
                                                tv.offset + C // 4,
                                                [tv.ap[0], [C // 2,
                                                            GPW],
                                                 [1, C // 4]]),
                                    op=Alu.add)
                                nc.vector.tensor_reduce(
                                    out=sc[:],
                                    in_=t2[:].rearrange(
                                        "p (t k) -> p t k", k=C // 4),
                                    axis=mybir.AxisListType.X,
                                    op=Alu.add)
                        if H == 1:
                            pv = wpool.tile([128, GPW], bf16,
                                            tag=f"pv{p}")
                            nc.scalar.activation(pv[:], sc[:], Act.Exp)
                            # lhsT ones column (denominator row of pso)
                            ov = onesB_t[:, :]
                            nc.vector.tensor_copy(
                                out=bass.AP(wpv.tensor, wpv.offset + HC,
                                            [wpv.ap[0], [WP, GPW],
                                             [1, 1]]),
                                in_=bass.AP(ov.tensor, ov.offset,
                                            [ov.ap[0], [0, GPW],
                                             [1, 1]]))
                            # scatter rhs = M * p (per-edge row scale)
                            mpv = mp_t[:, :]
                            Mtv = M_t[:, :]
                            pvv = pv[:, :]
                            nc.vector.tensor_tensor(
                                out=bass.AP(mpv.tensor,
                                            mpv.offset + p * SLOTS,
                                            [mpv.ap[0],
                                             [NPH * SLOTS, GPW],
                                             [1, SLOTS]]),
                                in0=bass.AP(Mtv.tensor,
                                            Mtv.offset + p * SLOTS,
                                            [Mtv.ap[0],
                                             [NPH * SLOTS, GPW],
                                             [1, SLOTS]]),
                                in1=bass.AP(pvv.tensor, pvv.offset,
                                            [pvv.ap[0], [1, GPW],
                                             [0, SLOTS]]),
                                op=Alu.mult)
                        else:
                            p_out = bass.AP(wpv.tensor, wpv.offset + HC,
                                            [wpv.ap[0], [WP, GPW],
                                             [1, H]])
                            nc.scalar.activation(p_out, sc[:], Act.Exp)
                            # w = xl * exp(sc); columns are c-major, so
                            # the p broadcast reads contiguous H-runs
                            w_out = bass.AP(wpv.tensor, wpv.offset,
                                            [wpv.ap[0], [WP, GPW],
                                             [H, C], [1, H]])
                            pe_b = bass.AP(wpv.tensor, wpv.offset + HC,
                                           [wpv.ap[0], [WP, GPW],
                                            [0, C], [1, H]])
                            nc.vector.tensor_tensor(
                                out=w_out,
                                in0=xl_sb[:].rearrange(
                                    "p (t c h) -> p t c h", c=C, h=H),
                                in1=pe_b, op=Alu.mult)
                        wps.append(wp_t)
                    # per slice: the 4 phase mms CONSECUTIVELY
                    # (start=True clears has_written bank-wide)
                    sc_rhs = mp_t if H == 1 else M_t
                    for j in range(GPW):
                        for p in range(NPH):
                            nc.tensor.matmul(
                                out=pso[:, j * SLOTS:(j + 1) * SLOTS],
                                lhsT=wps[p][:, j * WP:(j + 1) * WP],
                                rhs=sc_rhs[:, (j * NPH + p) * SLOTS:
                                           (j * NPH + p + 1) * SLOTS],
                                start=(p == 0), stop=(p == 3),
                                skip_group_check=True)

                    # finalize is emitted one window late (software
                    # pipeline): its cross-engine chain then never
                    # head-of-line-blocks the engine queues, since all
                    # its inputs were produced a full window earlier
                    if fin_prev is not None:
                        _emit_finalize(*fin_prev)
                    fin_prev = (pso, w)
                _emit_finalize(*fin_prev)

    nc.compile()
    return nc


def _run(nc, in_maps, trace=False):
    from concourse.bass_utils import run_bass_kernel_spmd
    return run_bass_kernel_spmd(nc, in_maps, core_ids=list(range(N_CORES)),
                                trace=trace)


def kernel(x, edge_index, edge_attr,
           Wl1, bl1, Wr1, br1, We1, att1, b1,
           Wl2, bl2, Wr2, br2, We2, att2, b2,
           _trace=False, _times=None):
    x = np.asarray(x, np.float32)
    per_core, G = _preprocess(np.asarray(edge_index),
                              np.asarray(edge_attr))
    S = G * SLOTS

    def bcast(v):
        v = np.asarray(v, np.float32).reshape(-1)
        return np.broadcast_to(v[None, :], (128, v.shape[0])).astype(BF16)

    def layer_inputs(xf, Wl, bl, Wr, br, We, att, b, D_IN, HC, H):
        C = HC // H

        def cmaj(a):
            # reorder feature columns (h,c) -> (c,h): the on-device score
            # reduce over c then runs on contiguous column halves
            a = np.asarray(a, np.float32)
            return a.reshape(*a.shape[:-1], H, C).swapaxes(-1, -2).reshape(
                *a.shape[:-1], HC)

        # weB [18, 128]: We/br/bl injector lhsT for the rhs_all build
        weB = np.zeros((D_EDGE + 2, 128), np.float32)
        weB[np.arange(D_EDGE), np.arange(D_EDGE)] = 1.0
        weB[np.arange(D_EDGE), 64 + np.arange(D_EDGE)] = 1.0
        weB[D_EDGE, 16:48] = 1.0
        weB[D_EDGE, 80:112] = 1.0
        weB[D_EDGE + 1, 48] = 1.0
        weB[D_EDGE + 1, 112] = 1.0
        webr = cmaj(np.concatenate(
            [np.asarray(We, np.float32),
             np.asarray(br, np.float32)[None, :],
             np.asarray(bl, np.float32)[None, :]], axis=0))
        # output bias absorbs bl (sum of alpha over a segment is 1)
        bout = cmaj(np.asarray(b, np.float32).reshape(-1)
                    + np.asarray(bl, np.float32).reshape(-1))
        att_f = cmaj(np.asarray(att, np.float32).reshape(-1))
        Wl = cmaj(Wl)
        Wr = cmaj(Wr)
        e8 = np.zeros((H, HC), np.float32)
        e8[np.arange(HC) % H, np.arange(HC)] = 1.0
        maps = []
        for c in range(N_CORES):
            pc = per_core[c]
            sn = pc["slot_node"]
            valid = sn >= 0
            # per-edge source features, transposed: [D_IN, G*512]
            xeT = np.ascontiguousarray(
                xf[pc["esrc"]].T).astype(BF16)
            # xT_slots [D_IN, G*128]: per group cols 16..48 and 80..112
            # hold the group's 32 slot features (two replicas), rest zero
            xs = np.zeros((G, 128, xf.shape[1]), np.float32)
            feats = np.zeros((G * SLOTS, xf.shape[1]), np.float32)
            feats[valid] = xf[sn[valid].astype(np.int64) + c * NPC]
            fg = feats.reshape(G, SLOTS, -1)
            xs[:, 16:48, :] = fg
            xs[:, 80:112, :] = fg
            xsT = np.ascontiguousarray(
                xs.reshape(G * 128, -1).T).astype(BF16)
            maps.append(dict(
                xeT=xeT, xT_slots=xsT,
                wl=np.asarray(Wl, np.float32).astype(BF16),
                wr=np.asarray(Wr, np.float32).astype(BF16),
                webr=webr.astype(BF16), weB=weB.astype(BF16),
                attB=bcast(att_f),
                biasC=bout.reshape(-1, 1).astype(np.float32),
                nbiasC=(-bout).reshape(-1, 1).astype(np.float32),
                onesB=np.ones((128, 1), BF16),
                exp8=e8.astype(BF16),
                ls=pc["ls"], M=pc["M"]))
        return maps

    def collect(res, width, H):
        # device rows are (c,h) c-major; un-permute back to (h,c)
        C = width // H
        out = np.zeros((N_NODES, width), np.float32)
        for c in range(N_CORES):
            sn = per_core[c]["slot_node"]
            valid = sn >= 0
            arr = np.asarray(res.results[c]["out_slots"]).astype(
                np.float32).T
            arr = arr.reshape(-1, C, H).swapaxes(1, 2).reshape(-1, width)
            out[sn[valid].astype(np.int64) + c * NPC] = arr[valid]
        return out

    nc1 = _build_layer(G, H1, C1, D_NODE, do_elu=True)
    res1 = _run(nc1, layer_inputs(x, Wl1, bl1, Wr1, br1, We1, att1, b1,
                                  D_NODE, H1 * C1, H1), trace=_trace)
    h = collect(res1, H1 * C1, H1)

    nc2 = _build_layer(G, 1, D_EMB, H1 * C1, do_elu=False)
    res2 = _run(nc2, layer_inputs(h, Wl2, bl2, Wr2, br2, We2, att2, b2,
                                  H1 * C1, D_EMB, 1), trace=_trace)
    out = collect(res2, D_EMB, 1)
    if _times is not None:
        _times.extend([res1.exec_time_ns, res2.exec_time_ns])
    return out


# revision 44
# speedup vs baseline: 1.1546x; 1.0095x over previous
"""GATv2 (2-layer, PyG-style self-loops) on 8 Trainium2 NeuronCores — bf16.

No dma_gather: the host stages per-edge source features x[src] in edge
order (pure layout), and the device projects them per-edge (lhsT=xeT
tile, rhs=Wl) straight into the score PSUM. This removes the SWDGE Q7
descriptor-generation serial bottleneck (~1ms/layer) and the table-build
prologue of the gather-based design.

Sharding: dst nodes split across 8 cores (12500 each); edges routed to the
core owning dst. Nodes packed into SLOT-GROUPS of <=32 slots and <=512
edges; each group's edges fill 4 tiles of 128 rows ("phases" p=row//128).
All feature columns are stored c-major (c,h) so the score reduce over c
runs as contiguous column halvings on the DVE.

Per phase-tile (bf16, PSUM fp32):
  psz  - 8 proj matmuls (lhsT=xeT 128-col tile, rhs=Wl) write xl per edge
         into PSUM (one start=True per tile; start clears has_written
         bank-wide), a scalar-engine Copy extracts xl to SBUF (for the
         message; H=1 writes straight into the wp slots), then 8 band
         matmuls accumulate ee+xr+biases: lhsT=[eaT(16);Mt(32);evalid]
         band, rhs=[We;br;bl;xr_g] (weconst + per-group xr matmuls).
  z    - LeakyReLU on the scalar engine.
  p    - z*att (DVE), halving-tree reduce, Exp on scalar engine.
  out  - lhsT=[xl*p | p] (H=1: [xl | 1] with p folded into the scatter
         rhs M*p) contracted with the one-hot M into a 512-slot PSUM
         window, 4 phases per slice consecutively.
Finalize (emitted one window late so its cross-engine chain never blocks
queue heads): reciprocal_approx_fast of the denominators (+eps for pad
slots), matmul-expanded to [HC,512], normalize, bias (+bl since
sum-alpha=1) and ELU via scalar-engine Relu/Exp with bias APs; output
stays [HC, S] (host transposes).
"""

import numpy as np
import ml_dtypes

BF16 = ml_dtypes.bfloat16

N_NODES = 100000
D_EDGE = 16
H1, C1 = 8, 8
D_NODE = 128
D_EMB = 64
NEG_SLOPE = 0.2
N_CORES = 8
NPC = N_NODES // N_CORES          # 12500 dst nodes per core
SLOTS = 32                        # slots per group
EPT = 128                         # edge rows per phase-tile
NPH = 4                           # tiles (phases) per group
GEDGE = NPH * EPT                 # 512 edge rows per group
GPW = 16                          # groups per psum window (512 slots)


def _preprocess(edge_index, edge_attr):
    src = np.asarray(edge_index[0], dtype=np.int64)
    dst = np.asarray(edge_index[1], dtype=np.int64)
    ea = np.asarray(edge_attr, dtype=np.float32)

    deg = np.bincount(dst, minlength=N_NODES).astype(np.float32)
    order0 = np.argsort(dst, kind="stable")
    ds = dst[order0]
    bnd0 = np.flatnonzero(np.diff(ds)) + 1
    starts0 = np.concatenate([[0], bnd0])
    ea_sum = np.zeros((N_NODES, D_EDGE), np.float32)
    ea_sum[ds[starts0]] = np.add.reduceat(ea[order0], starts0, axis=0)
    ea_mean = ea_sum / np.maximum(deg, 1.0)[:, None]

    loop = np.arange(N_NODES, dtype=np.int64)
    src2 = np.concatenate([src, loop])
    dst2 = np.concatenate([dst, loop])
    ea2 = np.concatenate([ea, ea_mean], axis=0)

    cores = []
    for c in range(N_CORES):
        lo = c * NPC
        m = (dst2 >= lo) & (dst2 < lo + NPC)
        cores.append((src2[m], dst2[m] - lo, ea2[m]))

    # --- per-core grouping: <=32 slots/group, <=512 edges/group (FFD) ---
    packed = []
    for (s_c, d_c, e_c) in cores:
        cnt = np.bincount(d_c, minlength=NPC).astype(np.int64)
        assert cnt.max() <= GEDGE
        grp = np.zeros(NPC, np.int64)
        slot = np.zeros(NPC, np.int64)
        order = np.argsort(-cnt, kind="stable")
        MAXOPEN = 64
        redges = np.zeros(0, np.int64)
        nslots = np.zeros(0, np.int64)
        gids = np.zeros(0, np.int64)
        ng = 0
        for n in order:
            cn = cnt[n]
            fits = (nslots < SLOTS) & (redges + cn <= GEDGE)
            j = int(np.argmax(fits)) if fits.any() else -1
            if j < 0:
                redges = np.concatenate([redges, [cn]])
                nslots = np.concatenate([nslots, [1]])
                gids = np.concatenate([gids, [ng]])
                grp[n] = ng
                slot[n] = 0
                ng += 1
                if len(gids) > MAXOPEN:
                    k = int(np.argmin(
                        (SLOTS - nslots) * GEDGE + (GEDGE - redges)))
                    redges = np.delete(redges, k)
                    nslots = np.delete(nslots, k)
                    gids = np.delete(gids, k)
            else:
                grp[n] = gids[j]
                slot[n] = nslots[j]
                redges[j] += cn
                nslots[j] += 1
        packed.append((s_c, d_c, e_c, grp, slot, ng))

    GREAL = max(p[-1] for p in packed)
    G = -(-GREAL // GPW) * GPW

    per_core = []
    for (s_c, d_c, e_c, grp, slot, _ng) in packed:
        ne = len(s_c)
        eg = grp[d_c]
        es = slot[d_c]
        o2 = np.lexsort((d_c, eg))
        eg2, es2 = eg[o2], es[o2]
        kb = np.flatnonzero(np.diff(eg2)) + 1
        kstarts = np.concatenate([[0], kb])
        r = np.arange(ne) - np.repeat(kstarts, np.diff(
            np.concatenate([kstarts, [ne]])))
        pos = eg2 * GEDGE + r                     # flat row in [G*512]
        NR = G * GEDGE

        esrc = np.zeros(NR, np.int64)
        esrc[pos] = s_c[o2]
        ea_rows = np.zeros((NR, D_EDGE), np.float32)
        ea_rows[pos] = e_c[o2]
        eslot = np.zeros(NR, np.int64)
        eslot[pos] = es2
        evalid = np.zeros(NR, np.float32)
        evalid[pos] = 1.0

        ea4 = ea_rows.reshape(G, NPH, EPT, D_EDGE)
        ev4 = evalid.reshape(G, NPH, EPT)
        rows = np.arange(NR)
        M4 = np.zeros((G, NPH, EPT, SLOTS), np.float32)
        M4[rows // GEDGE, (rows // EPT) % NPH, rows % EPT, eslot] = evalid

        # lhsT band stream [128, G, 2, 128]: band b=p%2 rows 64b..64b+64
        # hold phase p=2q+b at column-block q: rows +0:16 eaT, +16:48 Mt,
        # row +48 evalid (bl injector), rest zero
        ls4 = np.zeros((128, G, 2, EPT), np.float32)
        for p in range(NPH):
            b, q = p % 2, p // 2
            ls4[64 * b:64 * b + D_EDGE, :, q, :] = \
                ea4[:, p].transpose(2, 0, 1)
            ls4[64 * b + 16:64 * b + 16 + SLOTS, :, q, :] = \
                M4[:, p].transpose(2, 0, 1)
            ls4[64 * b + 48, :, q, :] = ev4[:, p]

        # M stream [128, G*4*SLOTS]
        Mflat = M4.transpose(2, 0, 1, 3).reshape(EPT, G * NPH * SLOTS)

        # slot -> node map
        slot_node = np.full(G * SLOTS, -1, np.int32)
        slot_node[grp * SLOTS + slot] = np.arange(NPC, dtype=np.int32)

        per_core.append(dict(
            ls=np.ascontiguousarray(
                ls4.reshape(128, G * 2 * EPT)).astype(BF16),
            M=np.ascontiguousarray(Mflat).astype(BF16),
            esrc=esrc, slot_node=slot_node))
    return per_core, G


def _build_layer(G, H, C, D_IN, do_elu):
    import concourse.bass as bass
    import concourse.mybir as mybir
    from concourse import bacc
    from concourse.tile import TileContext

    HC = H * C
    WP = HC + H
    S = G * SLOTS
    f32 = mybir.dt.float32
    bf16 = mybir.dt.bfloat16
    Alu = mybir.AluOpType
    Act = mybir.ActivationFunctionType
    NW = G // GPW

    nc = bacc.Bacc("TRN2", target_bir_lowering=False, debug=False,
                   num_devices=N_CORES)

    xeT_d = nc.dram_tensor("xeT", [D_IN, G * GEDGE], bf16,
                           kind="ExternalInput")
    xT_slots = nc.dram_tensor("xT_slots", [D_IN, G * 128], bf16,
                              kind="ExternalInput")
    wl = nc.dram_tensor("wl", [D_IN, HC], bf16, kind="ExternalInput")
    wr = nc.dram_tensor("wr", [D_IN, HC], bf16, kind="ExternalInput")
    webr = nc.dram_tensor("webr", [D_EDGE + 2, HC], bf16,
                          kind="ExternalInput")
    weB = nc.dram_tensor("weB", [D_EDGE + 2, 128], bf16,
                         kind="ExternalInput")
    attB = nc.dram_tensor("attB", [128, HC], bf16, kind="ExternalInput")
    biasC = nc.dram_tensor("biasC", [HC, 1], f32, kind="ExternalInput")
    nbiasC = nc.dram_tensor("nbiasC", [HC, 1], f32, kind="ExternalInput")
    onesB = nc.dram_tensor("onesB", [128, 1], bf16, kind="ExternalInput")
    exp8 = nc.dram_tensor("exp8", [H, HC], bf16, kind="ExternalInput")
    ls_d = nc.dram_tensor("ls", [128, G * 2 * EPT], bf16,
                          kind="ExternalInput")
    M_d = nc.dram_tensor("M", [128, G * NPH * SLOTS], bf16,
                         kind="ExternalInput")

    out_slots = nc.dram_tensor("out_slots", [HC, S], bf16,
                               kind="ExternalOutput")

    with TileContext(nc) as tc:
        with tc.tile_pool(name="const", bufs=1) as cpool:
            wl_t = cpool.tile([D_IN, HC], bf16)
            nc.sync.dma_start(wl_t[:], wl[:, :])
            wr_t = cpool.tile([D_IN, HC], bf16)
            nc.sync.dma_start(wr_t[:], wr[:, :])
            webr_t = cpool.tile([D_EDGE + 2, HC], bf16)
            nc.sync.dma_start(webr_t[:], webr[:, :])
            webr4_t = cpool.tile([D_EDGE + 2, 4, HC], bf16)
            wbv = webr_t[:, :]
            nc.vector.tensor_copy(
                out=webr4_t[:],
                in_=bass.AP(wbv.tensor, wbv.offset,
                            [wbv.ap[0], [0, 4], [1, HC]]))
            weB_t = cpool.tile([D_EDGE + 2, 128], bf16)
            nc.sync.dma_start(weB_t[:], weB[:, :])
            attB_t = cpool.tile([128, HC], bf16)
            nc.sync.dma_start(attB_t[:], attB[:, :])
            biasC_t = cpool.tile([HC, 1], f32)
            nc.sync.dma_start(biasC_t[:], biasC[:, :])
            nbiasC_t = cpool.tile([HC, 1], f32)
            nc.sync.dma_start(nbiasC_t[:], nbiasC[:, :])
            onesB_t = cpool.tile([128, 1], bf16)
            nc.sync.dma_start(onesB_t[:], onesB[:, :])
            exp8_t = cpool.tile([H, HC], bf16)
            nc.sync.dma_start(exp8_t[:], exp8[:, :])
            # att replicated GPW times for a flat contiguous zm multiply
            attW_t = cpool.tile([128, GPW * HC], bf16)
            ab0 = attB_t[:, :]
            nc.vector.tensor_copy(
                out=attW_t[:].rearrange("p (t c) -> p t c", c=HC),
                in_=bass.AP(ab0.tensor, ab0.offset,
                            [ab0.ap[0], [0, GPW], [1, HC]]))

            with tc.tile_pool(name="strm", bufs=2) as spool, \
                 tc.tile_pool(name="xe", bufs=2) as xpool, \
                 tc.tile_pool(name="rhs", bufs=2) as rpool, \
                 tc.tile_pool(name="work", bufs=2) as wpool, \
                 tc.tile_pool(name="bnc", bufs=2) as bpool, \
                 tc.tile_pool(name="zps", bufs=3, space="PSUM") as zps, \
                 tc.tile_pool(name="rps", bufs=2, space="PSUM") as rps, \
                 tc.tile_pool(name="xps", bufs=1, space="PSUM") as xps, \
                 tc.tile_pool(name="ops", bufs=2, space="PSUM") as ops:

                # static [We;br;bl] band content, built once:
                # rhs_all(window) = weconst + xr matmuls
                weconst = cpool.tile([128, 4, HC], bf16)
                prc = rps.tile([128, 4, HC], f32, space="PSUM", tag="pr")
                nc.tensor.matmul(
                    out=prc[:], lhsT=weB_t[:], rhs=webr4_t[:],
                    start=True, stop=True, skip_group_check=True)
                nc.vector.tensor_copy(out=weconst[:], in_=prc[:])

                def _emit_finalize(pso, w):
                    # transpose-free finalize in [hc-rows, slot-cols].
                    # +eps guards PAD slots (denominator exactly 0 there;
                    # approx_fast(0) is NaN and 0*NaN leaks via the mms)
                    s_eps = bpool.tile([H, 512], f32, tag="s")
                    nc.vector.tensor_scalar_add(
                        s_eps[:], pso[HC:HC + H, :], 1e-16)
                    rec = bpool.tile([H, 512], f32, tag="rec")
                    nc.vector.reciprocal_approx_fast(
                        out=rec[:], in_=s_eps[:])
                    rec_b = bpool.tile([H, 512], bf16, tag="recb")
                    nc.scalar.activation(rec_b[:], rec[:], Act.Copy)
                    recx_ps = xps.tile([HC, 512], f32, space="PSUM",
                                       tag="recx")
                    nc.tensor.matmul(
                        out=recx_ps[:], lhsT=exp8_t[:], rhs=rec_b[:],
                        start=True, stop=True, skip_group_check=True)
                    recx = bpool.tile([HC, 512], f32, tag="recxs")
                    nc.scalar.activation(recx[:], recx_ps[:], Act.Copy)
                    o = bpool.tile([HC, 512], f32, tag="o")
                    nc.vector.tensor_tensor(
                        out=o[:], in0=pso[0:HC, :], in1=recx[:],
                        op=Alu.mult)
                    ob = bpool.tile([HC, 512], bf16, tag="ob")
                    if do_elu:
                        # ELU(o+b) = relu(o+b) + exp(-relu(-(o+b))) - 1,
                        # biases applied via per-partition ACT bias APs
                        pos = bpool.tile([HC, 512], f32, tag="pos")
                        nc.scalar.activation(pos[:], o[:], Act.Relu,
                                             bias=biasC_t[:])
                        t1 = bpool.tile([HC, 512], f32, tag="t1")
                        nc.scalar.activation(t1[:], o[:], Act.Relu,
                                             scale=-1.0,
                                             bias=nbiasC_t[:])
                        en = bpool.tile([HC, 512], f32, tag="en")
                        nc.scalar.activation(en[:], t1[:], Act.Exp,
                                             scale=-1.0)
                        nc.vector.scalar_tensor_tensor(
                            out=ob[:], in0=en[:], scalar=-1.0,
                            in1=pos[:], op0=Alu.add, op1=Alu.add)
                    else:
                        nc.scalar.activation(ob[:], o[:], Act.Identity,
                                             bias=biasC_t[:])
                    nc.sync.dma_start(
                        out_slots[:, w * 512:(w + 1) * 512], ob[:])

                fin_prev = None
                for w in range(NW):
                    g0 = w * GPW
                    ls_t = spool.tile([128, GPW * 2 * EPT], bf16,
                                      tag="ls")
                    nc.sync.dma_start(
                        ls_t[:], ls_d[:, g0 * 2 * EPT:
                                      (g0 + GPW) * 2 * EPT])
                    M_t = spool.tile([128, GPW * NPH * SLOTS], bf16,
                                     tag="M")
                    nc.sync.dma_start(
                        M_t[:], M_d[:, g0 * NPH * SLOTS:
                                    (g0 + GPW) * NPH * SLOTS])
                    xe_t = xpool.tile([D_IN, GPW * GEDGE], bf16, tag="xe")
                    nc.scalar.dma_start(
                        xe_t[:], xeT_d[:, g0 * GEDGE:(g0 + GPW) * GEDGE])

                    xst = spool.tile([D_IN, GPW * 128], bf16, tag="xs")
                    nc.sync.dma_start(
                        xst[:], xT_slots[:, g0 * 128:(g0 + GPW) * 128])
                    rhs_all = None

                    pso = ops.tile([WP, GPW * SLOTS], f32, space="PSUM",
                                   tag="pso")
                    if H == 1:
                        # H=1: p is folded into the scatter rhs (M*p), so
                        # the lhsT is just [xl | 1] — xl copied straight
                        # into wp slots, no separate xl*p multiply
                        mp_t = wpool.tile([128, GPW * NPH * SLOTS], bf16,
                                          tag="mp")
                    wps = []
                    for p in range(NPH):
                        b64 = 64 * (p % 2)
                        q = p // 2
                        wp_t = wpool.tile([128, GPW * WP], bf16,
                                          tag=f"wp{p}")
                        wpv = wp_t[:, :]
                        if H > 1:
                            xl_sb = wpool.tile([128, GPW * HC], bf16,
                                               tag=f"xl{p}")
                        z0 = wpool.tile([128, GPW * HC], bf16,
                                        tag=f"z0{p}")
                        pszs = []
                        for h in range(2):
                            psz = zps.tile([128, 8 * HC], f32,
                                           space="PSUM", tag="psz")
                            # exactly ONE start=True per psz tile (the
                            # first mm): start=True clears has_written
                            # bank-wide, so later slices must use
                            # start=False and rely on per-element
                            # has_written (write-if-clear, else add)
                            for j in range(8):
                                gi = h * 8 + j
                                nc.tensor.matmul(
                                    out=psz[:, j * HC:(j + 1) * HC],
                                    lhsT=xe_t[:, (gi * NPH + p) * EPT:
                                              (gi * NPH + p + 1) * EPT],
                                    rhs=wl_t[:], start=(j == 0),
                                    stop=False,
                                    skip_group_check=True)
                            if H == 1:
                                xl_dst = bass.AP(wpv.tensor,
                                                 wpv.offset + h * 8 * WP,
                                                 [wpv.ap[0], [WP, 8],
                                                  [1, HC]])
                                if p % 2 == 1 and h == 1:
                                    # balance: DVE does 2 of 8 copies
                                    nc.vector.tensor_copy(
                                        out=xl_dst, in_=psz[:])
                                else:
                                    nc.scalar.activation(
                                        xl_dst, psz[:], Act.Copy)
                            elif p % 2 == 1 and h == 1:
                                nc.vector.tensor_copy(
                                    out=xl_sb[:, h * 8 * HC:
                                              (h + 1) * 8 * HC],
                                    in_=psz[:])
                            else:
                                nc.scalar.activation(
                                    xl_sb[:, h * 8 * HC:
                                          (h + 1) * 8 * HC],
                                    psz[:], Act.Copy)
                            pszs.append(psz)
                        if p == 0:
                            # rhs_all [128, GPW, HC]: per group two
                            # 64-row bands: +0:16 We, +16:48 xr slots,
                            # +48 bl. Built here (after phase-0 proj)
                            # so its DMA/add latency never FIFO-blocks
                            # independent proj work on the PE
                            rhs_all = rpool.tile([128, GPW, HC], bf16,
                                                 tag="r", space="SBUF")
                            for g4 in range(GPW // 4):
                                pr = rps.tile([128, 4, HC], f32,
                                              space="PSUM", tag="pr")
                                for jj in range(4):
                                    gi = g4 * 4 + jj
                                    nc.tensor.matmul(
                                        out=pr[:, jj, :],
                                        lhsT=xst[:, gi * 128:
                                                 (gi + 1) * 128],
                                        rhs=wr_t[:], start=(jj == 0),
                                        stop=(jj == 3),
                                        skip_group_check=True)
                                nc.vector.tensor_tensor(
                                    out=rhs_all[:, g4 * 4:
                                                (g4 + 1) * 4, :],
                                    in0=pr[:], in1=weconst[:],
                                    op=Alu.add)
                        for h in range(2):
                            psz = pszs[h]
                            for j in range(8):
                                gi = h * 8 + j
                                lcol = (gi * 2 + q) * EPT
                                nc.tensor.matmul(
                                    out=psz[:, j * HC:(j + 1) * HC],
                                    lhsT=ls_t[b64:b64 + 64,
                                              lcol:lcol + EPT],
                                    rhs=rhs_all[b64:b64 + 64, gi, :],
                                    start=False, stop=(j == 7),
                                    skip_group_check=True)
                            # z = LeakyReLU(s) on the scalar engine
                            nc.scalar.activation(
                                z0[:, h * 8 * HC:(h + 1) * 8 * HC],
                                psz[:], Act.Prelu, alpha=NEG_SLOPE)
                        zm = wpool.tile([128, GPW * HC], bf16,
                                        tag="zm")
                        nc.vector.tensor_tensor(
                            out=zm[:], in0=z0[:], in1=attW_t[:],
                            op=Alu.mult)
                        sc = wpool.tile([128, GPW * H], bf16,
                                        tag="sc")
                        with nc.allow_low_precision(
                                reason="bf16 score sum, |sc|~O(1)"):
                            if H > 1:
                                # columns are c-major (c,h): sum over c
                                # as a 3-step contiguous halving tree
                                # (full-rate DVE, no inner-8 penalty)
                                zv = zm[:, :]
                                t1 = wpool.tile([128, GPW * HC // 2],
                                                bf16, tag="t1r")
                                nc.vector.tensor_tensor(
                                    out=t1[:],
                                    in0=bass.AP(zv.tensor, zv.offset,
                                                [zv.ap[0], [HC, GPW],
                                                 [1, HC // 2]]),
                                    in1=bass.AP(zv.tensor,
                                                zv.offset + HC // 2,
                                                [zv.ap[0], [HC, GPW],
                                                 [1, HC // 2]]),
                                    op=Alu.add)
                                tv = t1[:, :]
                                t2 = wpool.tile([128, GPW * HC // 4],
                                                bf16, tag="t2r")
                                nc.vector.tensor_tensor(
                                    out=t2[:],
                                    in0=bass.AP(tv.tensor, tv.offset,
                                                [tv.ap[0], [HC // 2,
                                                            GPW],
                                                 [1, HC // 4]]),
                                    in1=bass.AP(tv.tensor,
                                                tv.offset + HC // 4,
                                                [tv.ap[0], [HC // 2,
                                                            GPW],
                                                 [1, HC // 4]]),
                                    op=Alu.add)
                                uv = t2[:, :]
                                nc.vector.tensor_tensor(
                                    out=sc[:],
                                    in0=bass.AP(uv.tensor, uv.offset,
                                                [uv.ap[0], [HC // 4,
                                                            GPW],
                                                 [1, HC // 8]]),
                                    in1=bass.AP(uv.tensor,
                                                uv.offset + HC // 8,
                                                [uv.ap[0], [HC // 4,
                                                            GPW],
                                                 [1, HC // 8]]),
                                    op=Alu.add)
                            else:
                                # H=1: two contiguous halvings, then a
                                # short 16-wide reduce
                                zv = zm[:, :]
                                t1 = wpool.tile([128, GPW * C // 2],
                                                bf16, tag="t1r")
                                nc.vector.tensor_tensor(
                                    out=t1[:],
                                    in0=bass.AP(zv.tensor, zv.offset,
                                                [zv.ap[0], [C, GPW],
                                                 [1, C // 2]]),
                                    in1=bass.AP(zv.tensor,
                                                zv.offset + C // 2,
                                                [zv.ap[0], [C, GPW],
                                                 [1, C // 2]]),
                                    op=Alu.add)
                                tv = t1[:, :]
                                t2 = wpool.tile([128, GPW * C // 4],
                                                bf16, tag="t2r")
                                nc.vector.tensor_tensor(
                                    out=t2[:],
                                    in0=bass.AP(tv.tensor, tv.offset,
                                                [tv.ap[0], [C // 2,
                                                            GPW],
                                                 [1, C // 4]]),
                                    in1=bass.AP(tv.tensor,
                                                tv.offset + C // 4,
                                                [tv.ap[0], [C // 2,
                                                            GPW],
                                                 [1, C // 4]]),
                                    op=Alu.add)
                                nc.vector.tensor_reduce(
                                    out=sc[:],
                                    in_=t2[:].rearrange(
                                        "p (t k) -> p t k", k=C // 4),
                                    axis=mybir.AxisListType.X,
                                    op=Alu.add)
                        if H == 1:
                            pv = wpool.tile([128, GPW], bf16,
                                            tag=f"pv{p}")
                            nc.scalar.activation(pv[:], sc[:], Act.Exp)
                            # lhsT ones column (denominator row of pso)
                            ov = onesB_t[:, :]
                            nc.vector.tensor_copy(
                                out=bass.AP(wpv.tensor, wpv.offset + HC,
                                            [wpv.ap[0], [WP, GPW],
                                             [1, 1]]),
                                in_=bass.AP(ov.tensor, ov.offset,
                                            [ov.ap[0], [0, GPW],
                                             [1, 1]]))
                            # scatter rhs = M * p (per-edge row scale)
                            mpv = mp_t[:, :]
                            Mtv = M_t[:, :]
                            pvv = pv[:, :]
                            nc.vector.tensor_tensor(
                                out=bass.AP(mpv.tensor,
                                            mpv.offset + p * SLOTS,
                                            [mpv.ap[0],
                                             [NPH * SLOTS, GPW],
                                             [1, SLOTS]]),
                                in0=bass.AP(Mtv.tensor,
                                            Mtv.offset + p * SLOTS,
                                            [Mtv.ap[0],
                                             [NPH * SLOTS, GPW],
                                             [1, SLOTS]]),
                                in1=bass.AP(pvv.tensor, pvv.offset,
                                            [pvv.ap[0], [1, GPW],
                                             [0, SLOTS]]),
                                op=Alu.mult)
                        else:
                            p_out = bass.AP(wpv.tensor, wpv.offset + HC,
                                            [wpv.ap[0], [WP, GPW],
                                             [1, H]])
                            nc.scalar.activation(p_out, sc[:], Act.Exp)
                            # w = xl * exp(sc); columns are c-major, so
                            # the p broadcast reads contiguous H-runs
                            w_out = bass.AP(wpv.tensor, wpv.offset,
                                            [wpv.ap[0], [WP, GPW],
                                             [H, C], [1, H]])
                            pe_b = bass.AP(wpv.tensor, wpv.offset + HC,
                                           [wpv.ap[0], [WP, GPW],
                                            [0, C], [1, H]])
                            nc.vector.tensor_tensor(
                                out=w_out,
                                in0=xl_sb[:].rearrange(
                                    "p (t c h) -> p t c h", c=C, h=H),
                                in1=pe_b, op=Alu.mult)
                        wps.append(wp_t)
                    # per slice: the 4 phase mms CONSECUTIVELY
                    # (start=True clears has_written bank-wide)
                    sc_rhs = mp_t if H == 1 else M_t
                    for j in range(GPW):
                        for p in range(NPH):
                            nc.tensor.matmul(
                                out=pso[:, j * SLOTS:(j + 1) * SLOTS],
                                lhsT=wps[p][:, j * WP:(j + 1) * WP],
                                rhs=sc_rhs[:, (j * NPH + p) * SLOTS:
                                           (j * NPH + p + 1) * SLOTS],
                                start=(p == 0), stop=(p == 3),
                                skip_group_check=True)

                    # finalize is emitted one window late (software
                    # pipeline): its cross-engine chain then never
                    # head-of-line-blocks the engine queues, since all
                    # its inputs were produced a full window earlier
                    if fin_prev is not None:
                        _emit_finalize(*fin_prev)
                    fin_prev = (pso, w)
                _emit_finalize(*fin_prev)

    nc.compile()
    return nc


def _run(nc, in_maps, trace=False):
    from concourse.bass_utils import run_bass_kernel_spmd
    return run_bass_kernel_spmd(nc, in_maps, core_ids=list(range(N_CORES)),
                                trace=trace)


def kernel(x, edge_index, edge_attr,
           Wl1, bl1, Wr1, br1, We1, att1, b1,
           Wl2, bl2, Wr2, br2, We2, att2, b2,
           _trace=False, _times=None):
    x = np.asarray(x, np.float32)
    per_core, G = _preprocess(np.asarray(edge_index),
                              np.asarray(edge_attr))
    S = G * SLOTS

    def bcast(v):
        v = np.asarray(v, np.float32).reshape(-1)
        return np.broadcast_to(v[None, :], (128, v.shape[0])).astype(BF16)

    def layer_inputs(xf, Wl, bl, Wr, br, We, att, b, D_IN, HC, H):
        C = HC // H

        def cmaj(a):
            # reorder feature columns (h,c) -> (c,h): the on-device score
            # reduce over c then runs on contiguous column halves
            a = np.asarray(a, np.float32)
            return a.reshape(*a.shape[:-1], H, C).swapaxes(-1, -2).reshape(
                *a.shape[:-1], HC)

        # weB [18, 128]: We/br/bl injector lhsT for the rhs_all build
        weB = np.zeros((D_EDGE + 2, 128), np.float32)
        weB[np.arange(D_EDGE), np.arange(D_EDGE)] = 1.0
        weB[np.arange(D_EDGE), 64 + np.arange(D_EDGE)] = 1.0
        weB[D_EDGE, 16:48] = 1.0
        weB[D_EDGE, 80:112] = 1.0
        weB[D_EDGE + 1, 48] = 1.0
        weB[D_EDGE + 1, 112] = 1.0
        webr = cmaj(np.concatenate(
            [np.asarray(We, np.float32),
             np.asarray(br, np.float32)[None, :],
             np.asarray(bl, np.float32)[None, :]], axis=0))
        # output bias absorbs bl (sum of alpha over a segment is 1)
        bout = cmaj(np.asarray(b, np.float32).reshape(-1)
                    + np.asarray(bl, np.float32).reshape(-1))
        att_f = cmaj(np.asarray(att, np.float32).reshape(-1))
        Wl = cmaj(Wl)
        Wr = cmaj(Wr)
        e8 = np.zeros((H, HC), np.float32)
        e8[np.arange(HC) % H, np.arange(HC)] = 1.0
        maps = []
        for c in range(N_CORES):
            pc = per_core[c]
            sn = pc["slot_node"]
            valid = sn >= 0
            # per-edge source features, transposed: [D_IN, G*512]
            xeT = np.ascontiguousarray(
                xf[pc["esrc"]].T).astype(BF16)
            # xT_slots [D_IN, G*128]: per group cols 16..48 and 80..112
            # hold the group's 32 slot features (two replicas), rest zero
            xs = np.zeros((G, 128, xf.shape[1]), np.float32)
            feats = np.zeros((G * SLOTS, xf.shape[1]), np.float32)
            feats[valid] = xf[sn[valid].astype(np.int64) + c * NPC]
            fg = feats.reshape(G, SLOTS, -1)
            xs[:, 16:48, :] = fg
            xs[:, 80:112, :] = fg
            xsT = np.ascontiguousarray(
                xs.reshape(G * 128, -1).T).astype(BF16)
            maps.append(dict(
                xeT=xeT, xT_slots=xsT,
                wl=np.asarray(Wl, np.float32).astype(BF16),
                wr=np.asarray(Wr, np.float32).astype(BF16),
                webr=webr.astype(BF16), weB=weB.astype(BF16),
                attB=bcast(att_f),
                biasC=bout.reshape(-1, 1).astype(np.float32),
                nbiasC=(-bout).reshape(-1, 1).astype(np.float32),
                onesB=np.ones((128, 1), BF16),
                exp8=e8.astype(BF16),
                ls=pc["ls"], M=pc["M"]))
        return maps

    def collect(res, width, H):
        # device rows are (c,h) c-major; un-permute back to (h,c)
        C = width // H
        out = np.zeros((N_NODES, width), np.float32)
        for c in range(N_CORES):
            sn = per_core[c]["slot_node"]
            valid = sn >= 0
            arr = np.asarray(res.results[c]["out_slots"]).astype(
                np.float32).T
            arr = arr.reshape(-1, C, H).swapaxes(1, 2).reshape(-1, width)
            out[sn[valid].astype(np.int64) + c * NPC] = arr[valid]
        return out

    nc1 = _build_layer(G, H1, C1, D_NODE, do_elu=True)
    res1 = _run(nc1, layer_inputs(x, Wl1, bl1, Wr1, br1, We1, att1, b1,
                                  D_NODE, H1 * C1, H1), trace=_trace)
    h = collect(res1, H1 * C1, H1)

    nc2 = _build_layer(G, 1, D_EMB, H1 * C1, do_elu=False)
    res2 = _run(nc2, layer_inputs(h, Wl2, bl2, Wr2, br2, We2, att2, b2,
                                  H1 * C1, D_EMB, 1), trace=_trace)
    out = collect(res2, D_EMB, 1)
    if _times is not None:
        _times.extend([res1.exec_time_ns, res2.exec_time_ns])
    return out
